# revision 38
# baseline (speedup 1.0000x reference)
"""Trainium2 Bass kernel for EnhancedFastKANLayer.

Reference computation (B=16384, D=O=512, G=8 grids):
    x_norm = (x - mean) * rsqrt(var + eps) * gamma + beta          # BN inference
    basis[b,d,g] = exp(-((x_norm[b,d] - grid[g]) / denom)^2)       # RBF expansion
    out = basis.reshape(B, D*G) @ W_spline + b_spline
        + relu(x) @ W_base + b_base + x

Strategy:
  - Data parallel: batch 16384 sharded 8 ways (2048 rows/core); weights
    replicated. No collectives.
  - All on-chip compute happens in the transposed layout [feature, batch]:
    the output is produced as out_T [O, B_shard] and transposed back on the
    host. This makes BN/basis per-partition-scalar ops and lets the spline
    matmul consume basis tiles directly as the moving operand.
  - RBF via ScalarE Derivative_Erf: d/dx erf(x) = 2/sqrt(pi)*exp(-x^2), so
    basis_g = sqrt(pi)/2 * Derivative_Erf(u - c_g) -- ONE ACT op per grid
    (bias supplies -c_g), with the sqrt(pi)/2 constant folded into W_spline
    on the host.  HW-probed: rel err 1.3e-5, saturates cleanly to 0 for
    |x| > 6, no NaN/Inf out to |x|=24.
  - x is pre-cast to fp16 AND pre-transposed to [D, B_shard] on the host:
    fp16 keeps the BN input error at 2^-11 relative, and host-side
    transposition means every device DMA is a plain contiguous copy.
  - W_spline is pre-reordered on the host to K-order (dt, g, d_in) matching
    the order basis tiles are produced on chip, cast to fp16.
  - Matmul: out_T[o_sub, b] accumulates 36 matmuls per PSUM tile:
    32 spline K-chunks + 4 relu(x)@W_base K-chunks.  The "+ x" residual and
    the output bias are NOT matmuls: the epilogue is a single DVE
    tensor_tensor  out = psum + xb,  where xb = x_T + bias was precomputed
    on idle DVE cycles during the main matmul stream.  (A K=128 identity
    matmul costs the same 512 PE columns as a real K-chunk -- 1/37 of all
    PE time -- so it is strictly cheaper on the DVE.)
  - Startup choreography (trace-driven): the framework preamble ends ~7us;
    DMA queues move bytes from ~8.5us, and the ~358 GB/s per-core HBM
    bandwidth is SHARED across the three usable queues (sync HWDGE, scalar
    HWDGE, gpsimd SWDGE), so whatever streams first starves the rest.
    Order of battle:
      sync:    params (tiny) -> xt[dt0] ch0/ch1 -> W[6:32]+Wb (needed from
               ~24us) -> xt[dt3] pair (needed ~40us)
      gpsimd:  W[0:2], W[2:6] only (first PE block's weights, ~0.8 MB)
      scalar:  xt[dt1] pair / xt[dt2] pair, issued BETWEEN basis ACT groups
               (the ACT stream itself paces them; each lands well before
               its d-tile's basis is produced)
    so the startup-critical bytes (params + xt[dt0] + W[0:6]) have the HBM
    to themselves and the first real matmul can fire ~11.5-12.5us.
  - All matmul blocks are kc-outer (all 8 PSUM tiles at kc, then kc+1):
    the PE consumes one new basis tile per ~1.7us instead of one per
    ~0.2us burst, so it never outruns the ACT engine or the W stream.
  - Warmup matmuls (vs a memset zeros tile -- no identity needed) start as
    soon as the DVE memset lands (~7us), ramping the HAM clock throttle
    (full clock needs ~3us of sustained PE activity, and a PE idle gap can
    drop it back) while the first basis tiles are made.
  - Output is written fp16 (2^-11 relative is far inside the error budget),
    halving the tail DMA drain after the last matmul.
"""

import numpy as np
from contextlib import ExitStack

import concourse.bass as bass
import concourse.tile as tile
from concourse import bacc, mybir
from concourse._compat import with_exitstack
from concourse.bass_utils import run_bass_kernel_spmd

N_CORES = 8
BATCH, IN_DIM, OUT_DIM, G = 16384, 512, 512, 8
B_SHARD = BATCH // N_CORES          # 2048
B_CHUNK = 1024                      # batch columns processed per chunk
GRID_MIN, GRID_MAX, BN_EPS = -2.0, 2.0, 1e-3
DENOM = (GRID_MAX - GRID_MIN) / G   # 0.5
N_DT = IN_DIM // 128                # 4 d-tiles
K_SPLINE = N_DT * G                 # 32 spline K-chunks
K_BASE = N_DT                       # 4 base K-chunks
N_OSUB = OUT_DIM // 128             # 4 output partition tiles

F32 = mybir.dt.float32
F16 = mybir.dt.float16


def _grid_consts():
    grid = np.linspace(GRID_MIN, GRID_MAX, G, dtype=np.float32)
    c = (grid / np.float32(DENOM)).astype(np.float32)        # grid in u-units
    return c


@with_exitstack
def _body(ctx, tc, x16t, w_sp, w_b, params, out_t, b_shard, b_chunk):
    nc = tc.nc
    n_chunks = b_shard // b_chunk
    n_bh = b_chunk // 512            # 512-wide moving-operand slices
    k_total = K_SPLINE + K_BASE      # 36 K-chunks per PSUM tile
    KB = 6                           # K-chunks per PE block
    K_LAST = 30                      # kc >= K_LAST run per-bank, fused w/ epilogue
    W_SLICE = 8                      # spline weight K-chunks per SWDGE DMA

    const_pool = ctx.enter_context(tc.tile_pool(name="const", bufs=1))
    w_pool = ctx.enter_context(tc.tile_pool(name="w", bufs=1))
    xt_pool = ctx.enter_context(tc.tile_pool(name="xt", bufs=2 * N_DT))
    u_pool = ctx.enter_context(tc.tile_pool(name="u", bufs=3))
    # spline basis tiles: 32 per chunk stay resident through the chunk's
    # matmul phase; extra slots let the next chunk's production run ahead.
    basis_pool = ctx.enter_context(tc.tile_pool(name="basis", bufs=K_SPLINE + 12))
    relu_pool = ctx.enter_context(tc.tile_pool(name="relu", bufs=2 * N_DT))
    xb_pool = ctx.enter_context(tc.tile_pool(name="xb", bufs=2 * N_OSUB * 2))
    psum_pool = ctx.enter_context(
        tc.tile_pool(name="psum", bufs=8, space="PSUM"))
    out_pool = ctx.enter_context(tc.tile_pool(name="outs", bufs=4))

    # ---- warmup stationary + ACT table warm.  zeros via DVE memset is
    # ready ~2us before the old identity construction (gpsimd affine_select
    # behind the params DMA on the same engine), so the PE clock ramp
    # starts that much earlier ----
    zeros = const_pool.tile([128, 128], F16)
    nc.vector.memset(zeros, 0.0)
    scratch = const_pool.tile([128, 1], F16)
    nc.vector.memset(scratch, 0.0)
    # walrus inserts the ACT_TABLE_LOAD before this first ACTIVATE so it
    # overlaps the input DMAs
    nc.scalar.activation(out=scratch, in_=scratch,
                         func=mybir.ActivationFunctionType.Derivative_Erf)

    # ---- DMA order of battle (see module docstring).  xt[dt][ch] tiles for
    # BOTH chunks are allocated up front; dt0 + dt3 ride the sync queue,
    # dt1/dt2 are issued from the scalar engine inside the producer loop. ----
    params_sb = const_pool.tile([128, N_DT + N_DT + N_OSUB + G], F32)
    uscale_sb = params_sb[:, 0:N_DT]
    ushift_sb = params_sb[:, N_DT:2 * N_DT]
    bias_sb = params_sb[:, 2 * N_DT:2 * N_DT + N_OSUB]
    negc = params_sb[:, 2 * N_DT + N_OSUB:]

    xts = [[xt_pool.tile([128, b_chunk], F16, tag="xt",
                         name=f"xt{dt}_{ch}")
            for ch in range(n_chunks)] for dt in range(N_DT)]

    def xt_dma(eng, dt, ch):
        eng.dma_start(
            out=xts[dt][ch],
            in_=x16t[dt * 128:(dt + 1) * 128,
                     ch * b_chunk:(ch + 1) * b_chunk])

    # The two startup-critical 128-packet loads ride the fast sync queue
    # back-to-back: xt(dt0,ch0) lands ~10.2us (gates u -> basis ~11.5),
    # W[0:2] lands ~12.2us (gates the first matmul).  params (SWDGE
    # coalesces its tiny rows, done ~9.9) and W[2:6] (needed ~16us) go on
    # the SWDGE queue concurrently.
    # sync carries ONLY the two startup-critical 128-packet loads: a third
    # in-flight descriptor interleaves into the second's tail and drags its
    # completion by ~1.5us (observed).
    xt_dma(nc.sync, 0, 0)
    w_tile = w_pool.tile([128, K_SPLINE, OUT_DIM], F16)
    # W[0:2] partition-split across sync (lower half, behind xt0ch0) and
    # scalar (upper half, its first descriptor): both land ~11.3us, just
    # after basis00 (~11.9), pulling the first matmul ~0.3us earlier.
    nc.sync.dma_start(out=w_tile[0:64, 0:2, :], in_=w_sp[0:64, 0:2, :])
    # Everything else streams on SWDGE in order of first need (params ~10.6,
    # kc2 ~16, kc6 ~23, u(dt1) ~24, kc14 ~31, u(dt2) ~35, ...).  Keeping the
    # xt[1..2] loads here (instead of scalar-engine descs) frees ~1.3us of
    # the ACT stream, which is the critical path through the dt0->dt1
    # basis handoff.
    wb_tile = w_pool.tile([128, K_BASE, OUT_DIM], F16)
    nc.gpsimd.dma_start(out=params_sb, in_=params)
    nc.gpsimd.dma_start(out=w_tile[:, 2:KB, :], in_=w_sp[:, 2:KB, :])
    nc.gpsimd.dma_start(out=w_tile[:, 6:14, :], in_=w_sp[:, 6:14, :])
    xt_dma(nc.gpsimd, 1, 0)
    xt_dma(nc.gpsimd, 1, 1)
    nc.gpsimd.dma_start(out=w_tile[:, 14:22, :], in_=w_sp[:, 14:22, :])
    xt_dma(nc.gpsimd, 2, 0)
    xt_dma(nc.gpsimd, 2, 1)
    nc.gpsimd.dma_start(out=w_tile[:, 22:30, :], in_=w_sp[:, 22:30, :])
    nc.gpsimd.dma_start(out=w_tile[:, 30:32, :], in_=w_sp[:, 30:32, :])
    nc.gpsimd.dma_start(out=wb_tile, in_=w_b)
    xt_dma(nc.gpsimd, 3, 0)
    xt_dma(nc.gpsimd, 3, 1)
    # scalar queue: W[0:2] upper half first, then xt(dt0,ch1).  Emitted
    # after the scratch ACTIVATE: a desc before the first ACT triggers a
    # second ACT_TABLE_LOAD (observed, +1.5us of ACT stream).
    nc.scalar.dma_start(out=w_tile[64:128, 0:2, :], in_=w_sp[64:128, 0:2, :])
    xt_dma(nc.scalar, 0, 1)

    def emit_producers(ch):
        relus, basis = [], []
        for dt in range(N_DT):
            u = u_pool.tile([128, b_chunk], F32, tag="u")
            # chunk 0 / dt 0 is on the startup critical path: emit u and the
            # basis ACTs in halves so the first matmul's inputs exist as
            # soon as possible after the xt(dt0) DMA lands.
            halved = (ch == 0 and dt == 0)
            if halved:
                for h in range(0, b_chunk, 512):
                    nc.vector.tensor_scalar(
                        out=u[:, h:h + 512], in0=xts[dt][ch][:, h:h + 512],
                        scalar1=uscale_sb[:, dt:dt + 1],
                        scalar2=ushift_sb[:, dt:dt + 1],
                        op0=mybir.AluOpType.mult, op1=mybir.AluOpType.add,
                    )
            else:
                nc.vector.tensor_scalar(
                    out=u, in0=xts[dt][ch],
                    scalar1=uscale_sb[:, dt:dt + 1],
                    scalar2=ushift_sb[:, dt:dt + 1],
                    op0=mybir.AluOpType.mult, op1=mybir.AluOpType.add,
                )
            rl = relu_pool.tile([128, b_chunk], F16, tag="relu")
            nc.vector.tensor_scalar_max(out=rl, in0=xts[dt][ch], scalar1=0.0)
            relus.append(rl)
            for g in range(G):
                bt = basis_pool.tile([128, b_chunk], F16, tag="basis")
                # basis_g = sqrt(pi)/2 * d/dx erf(u - c_g); constant folded
                # into W_spline host-side.
                if halved:
                    for h in range(0, b_chunk, 512):
                        nc.scalar.activation(
                            out=bt[:, h:h + 512], in_=u[:, h:h + 512],
                            func=mybir.ActivationFunctionType.Derivative_Erf,
                            bias=negc[:, g:g + 1],
                        )
                else:
                    nc.scalar.activation(
                        out=bt, in_=u,
                        func=mybir.ActivationFunctionType.Derivative_Erf,
                        bias=negc[:, g:g + 1],
                    )
                basis.append(bt)
        return relus, basis

    def emit_xb(ch):
        # xb[osub,bh] = x_T + bias  (per-partition scalar add on fp16, 2x
        # DVE rate) -- consumed by the single-op epilogue.  Runs on DVE
        # cycles that are otherwise idle during the matmul stream; emitted
        # just before the chunk's final block so waiting on late xt tiles
        # never stalls the next chunk's u production.
        xbs = []
        for osub in range(N_OSUB):
            for bh in range(n_bh):
                xb = xb_pool.tile([128, 512], F16, tag="xb")
                nc.vector.tensor_scalar_add(
                    out=xb,
                    in0=xts[osub][ch][:, bh * 512:(bh + 1) * 512],
                    scalar1=bias_sb[:, osub:osub + 1])
                xbs.append(xb)
        return xbs

    def operands(kc, osub, relus, basis):
        if kc < K_SPLINE:
            return w_tile[:, kc, osub * 128:(osub + 1) * 128], basis[kc]
        dt = kc - K_SPLINE
        return wb_tile[:, dt, osub * 128:(osub + 1) * 128], relus[dt]

    def emit_main_blocks(ch, psums, relus, basis):
        # kc-outer: the PE touches a new basis tile only every
        # N_OSUB*n_bh matmuls (~1.7us), so late producers don't stall it.
        for kb in range(0, K_LAST, KB):
            for kc in range(kb, kb + KB):
                for osub in range(N_OSUB):
                    for bh in range(n_bh):
                        lhsT, rhs = operands(kc, osub, relus, basis)
                        nc.tensor.matmul(
                            psums[osub * n_bh + bh], lhsT=lhsT,
                            rhs=rhs[:, bh * 512:(bh + 1) * 512],
                            start=(kc == 0), stop=False)

    def emit_final_block(ch, psums, relus, basis, xbs):
        b0 = ch * b_chunk
        for osub in range(N_OSUB):
            for bh in range(n_bh):
                ps = psums[osub * n_bh + bh]
                for kc in range(K_LAST, k_total):
                    lhsT, rhs = operands(kc, osub, relus, basis)
                    nc.tensor.matmul(
                        ps, lhsT=lhsT, rhs=rhs[:, bh * 512:(bh + 1) * 512],
                        start=False, stop=(kc == k_total - 1))
                ot = out_pool.tile([128, 512], F16, tag="ot")
                xb = xbs[osub * n_bh + bh]
                last_tile = (ch == n_chunks - 1 and osub == N_OSUB - 1
                             and bh == n_bh - 1)
                orow = out_t[osub * 128:(osub + 1) * 128,
                             b0 + bh * 512:b0 + (bh + 1) * 512]
                if last_tile:
                    # the very last tile is the post-stream tail: halve the
                    # epilogue and fan the two stores across the scalar and
                    # sync queues so DVE time and descriptor time overlap
                    for h, eng in ((0, nc.scalar), (256, nc.sync)):
                        nc.vector.tensor_tensor(
                            out=ot[:, h:h + 256], in0=ps[:, h:h + 256],
                            in1=xb[:, h:h + 256], op=mybir.AluOpType.add)
                        eng.dma_start(out=orow[:, h:h + 256],
                                      in_=ot[:, h:h + 256])
                else:
                    nc.vector.tensor_tensor(
                        out=ot, in0=ps, in1=xb, op=mybir.AluOpType.add)
                    # last chunk's stores go out on the (by then idle)
                    # scalar queue so the final store doesn't wait behind
                    # the sync queue's drain of the earlier tiles
                    eng = nc.scalar if ch == n_chunks - 1 else nc.sync
                    eng.dma_start(out=orow, in_=ot)

    # Emission order keeps each engine's in-order stream free of
    # cross-chunk serialization: chunk ch+1's DVE/ACT producer ops are
    # emitted BEFORE chunk ch's final block + epilogue (which wait on ch's
    # last matmuls).
    def alloc_psums(ch):
        return [psum_pool.tile([128, 512], F32, tag="ps", name=f"ps{ch}_{i}")
                for i in range(N_OSUB * n_bh)]

    psums0 = alloc_psums(0)
    # PE warm-up: dependency-free matmuls into psums0[0] start the HAM clock
    # ramp (~3us of sustained PE activity to reach full clock) while the
    # first basis tiles are still being produced; the real kc==0 matmul has
    # start=True, which resets the bank, so the junk never reaches the
    # output.  Sized to bridge from zeros-ready (~7us) to first-basis-ready
    # (~12us) at the mid-pstate rate: an undershoot idles the PE and can
    # drop the HAM clock back to half speed (observed), so err long.
    N_WARM = 46
    for j in range(N_WARM):
        nc.tensor.matmul(psums0[0][:, 0:128], lhsT=zeros, rhs=zeros,
                         start=(j == 0), stop=(j == N_WARM - 1))

    prod = emit_producers(0)
    psums = psums0
    for ch in range(n_chunks):
        emit_main_blocks(ch, psums, *prod)
        cur_prod, cur_psums = prod, psums
        if ch + 1 < n_chunks:
            prod = emit_producers(ch + 1)
            psums = alloc_psums(ch + 1)
        emit_final_block(ch, cur_psums, *cur_prod, emit_xb(ch))


def build_program(b_shard=B_SHARD, b_chunk=B_CHUNK):
    nc = bacc.Bacc("TRN2", target_bir_lowering=False, debug=False,
                   num_devices=N_CORES)
    x16t = nc.dram_tensor("x16t", [IN_DIM, b_shard], F16,
                          kind="ExternalInput").ap()
    w_sp = nc.dram_tensor("w_sp", [128, K_SPLINE, OUT_DIM], F16,
                          kind="ExternalInput").ap()
    w_b = nc.dram_tensor("w_base", [128, K_BASE, OUT_DIM], F16,
                         kind="ExternalInput").ap()
    n_par = 2 * N_DT + N_OSUB + G
    params = nc.dram_tensor("params", [128, n_par], F32,
                            kind="ExternalInput").ap()
    out_t = nc.dram_tensor("out_t", [OUT_DIM, b_shard], F16,
                           kind="ExternalOutput").ap()
    with tile.TileContext(nc) as tc:
        _body(tc, x16t, w_sp, w_b, params, out_t, b_shard, b_chunk)
    nc.compile()
    return nc


def make_in_maps(x, gamma, beta, moving_mean, moving_var, W_spline, b_spline,
                 W_base, b_base, n_cores=N_CORES):
    """Host-side preprocessing + per-core input shards."""
    x = np.asarray(x, dtype=np.float32)
    gamma = np.asarray(gamma, dtype=np.float32)
    beta = np.asarray(beta, dtype=np.float32)
    moving_mean = np.asarray(moving_mean, dtype=np.float32)
    moving_var = np.asarray(moving_var, dtype=np.float32)
    W_spline = np.asarray(W_spline, dtype=np.float32)
    W_base = np.asarray(W_base, dtype=np.float32)
    b_spline = np.asarray(b_spline, dtype=np.float32)
    b_base = np.asarray(b_base, dtype=np.float32)

    scale = gamma / np.sqrt(moving_var + np.float32(BN_EPS))
    shift = beta - moving_mean * scale
    uscale = (scale / np.float32(DENOM)).astype(np.float32)
    ushift = (shift / np.float32(DENOM)).astype(np.float32)

    x16t = np.ascontiguousarray(x.T.astype(np.float16))  # [D, B]
    # K-order on chip is (dt, g, d_in): kc = dt*8+g covers d in
    # [dt*128, (dt+1)*128) at grid g.  W_spline rows are (d, g)-ordered.
    w_r = (W_spline.reshape(N_DT, 128, G, OUT_DIM)
           .transpose(0, 2, 1, 3)            # (dt, g, d_in, o)
           .reshape(K_SPLINE, 128, OUT_DIM)
           .transpose(1, 0, 2))              # (d_in, kc, o)
    w_sp = np.ascontiguousarray(w_r * np.float32(np.sqrt(np.pi) / 2.0)
                               ).astype(np.float16)
    w_b = np.ascontiguousarray(
        W_base.reshape(K_BASE, 128, OUT_DIM).transpose(1, 0, 2)
    ).astype(np.float16)
    bias_o = (b_spline + b_base).astype(np.float32)
    c = _grid_consts()
    params = np.empty((128, 2 * N_DT + N_OSUB + G), np.float32)
    params[:, 0:N_DT] = uscale.reshape(N_DT, 128).T
    params[:, N_DT:2 * N_DT] = ushift.reshape(N_DT, 128).T
    params[:, 2 * N_DT:2 * N_DT + N_OSUB] = bias_o.reshape(N_OSUB, 128).T
    params[:, 2 * N_DT + N_OSUB:] = -c[None, :]

    b_shard = x.shape[0] // n_cores
    return [
        {
            "x16t": np.ascontiguousarray(
                x16t[:, ci * b_shard:(ci + 1) * b_shard]),
            "w_sp": w_sp,
            "w_base": w_b,
            "params": params,
        }
        for ci in range(n_cores)
    ]


_PROGRAM = None


def kernel(x, gamma, beta, moving_mean, moving_var, W_spline, b_spline,
           W_base, b_base):
    global _PROGRAM
    if _PROGRAM is None:
        _PROGRAM = build_program()
    in_maps = make_in_maps(x, gamma, beta, moving_mean, moving_var,
                           W_spline, b_spline, W_base, b_base)
    res = run_bass_kernel_spmd(_PROGRAM, in_maps, core_ids=list(range(N_CORES)))
    out = np.concatenate(
        [np.ascontiguousarray(res.results[ci]["out_t"].T)
         for ci in range(N_CORES)], axis=0)
    return out.astype(np.float32)


# revision 39
# speedup vs baseline: 1.0019x; 1.0019x over previous
"""Trainium2 Bass kernel for EnhancedFastKANLayer.

Reference computation (B=16384, D=O=512, G=8 grids):
    x_norm = (x - mean) * rsqrt(var + eps) * gamma + beta          # BN inference
    basis[b,d,g] = exp(-((x_norm[b,d] - grid[g]) / denom)^2)       # RBF expansion
    out = basis.reshape(B, D*G) @ W_spline + b_spline
        + relu(x) @ W_base + b_base + x

Strategy:
  - Data parallel: batch 16384 sharded 8 ways (2048 rows/core); weights
    replicated. No collectives.
  - All on-chip compute happens in the transposed layout [feature, batch]:
    the output is produced as out_T [O, B_shard] and transposed back on the
    host. This makes BN/basis per-partition-scalar ops and lets the spline
    matmul consume basis tiles directly as the moving operand.
  - RBF via ScalarE Derivative_Erf: d/dx erf(x) = 2/sqrt(pi)*exp(-x^2), so
    basis_g = sqrt(pi)/2 * Derivative_Erf(u - c_g) -- ONE ACT op per grid
    (bias supplies -c_g), with the sqrt(pi)/2 constant folded into W_spline
    on the host.  HW-probed: rel err 1.3e-5, saturates cleanly to 0 for
    |x| > 6, no NaN/Inf out to |x|=24.
  - x is pre-cast to fp16 AND pre-transposed to [D, B_shard] on the host:
    fp16 keeps the BN input error at 2^-11 relative, and host-side
    transposition means every device DMA is a plain contiguous copy.
  - W_spline is pre-reordered on the host to K-order (dt, g, d_in) matching
    the order basis tiles are produced on chip, cast to fp16.
  - Matmul: out_T[o_sub, b] accumulates 36 matmuls per PSUM tile:
    32 spline K-chunks + 4 relu(x)@W_base K-chunks.  The "+ x" residual and
    the output bias are NOT matmuls: the epilogue is a single DVE
    tensor_tensor  out = psum + xb,  where xb = x_T + bias was precomputed
    on idle DVE cycles during the main matmul stream.  (A K=128 identity
    matmul costs the same 512 PE columns as a real K-chunk -- 1/37 of all
    PE time -- so it is strictly cheaper on the DVE.)
  - Startup choreography (trace-driven): the framework preamble ends ~7us;
    DMA queues move bytes from ~8.5us, and the ~358 GB/s per-core HBM
    bandwidth is SHARED across the three usable queues (sync HWDGE, scalar
    HWDGE, gpsimd SWDGE), so whatever streams first starves the rest.
    Order of battle:
      sync:    params (tiny) -> xt[dt0] ch0/ch1 -> W[6:32]+Wb (needed from
               ~24us) -> xt[dt3] pair (needed ~40us)
      gpsimd:  W[0:2], W[2:6] only (first PE block's weights, ~0.8 MB)
      scalar:  xt[dt1] pair / xt[dt2] pair, issued BETWEEN basis ACT groups
               (the ACT stream itself paces them; each lands well before
               its d-tile's basis is produced)
    so the startup-critical bytes (params + xt[dt0] + W[0:6]) have the HBM
    to themselves and the first real matmul can fire ~11.5-12.5us.
  - All matmul blocks are kc-outer (all 8 PSUM tiles at kc, then kc+1):
    the PE consumes one new basis tile per ~1.7us instead of one per
    ~0.2us burst, so it never outruns the ACT engine or the W stream.
  - Warmup matmuls (vs a memset zeros tile -- no identity needed) start as
    soon as the DVE memset lands (~7us), ramping the HAM clock throttle
    (full clock needs ~3us of sustained PE activity, and a PE idle gap can
    drop it back) while the first basis tiles are made.
  - Output is written fp16 (2^-11 relative is far inside the error budget),
    halving the tail DMA drain after the last matmul.
"""

import numpy as np
from contextlib import ExitStack

import concourse.bass as bass
import concourse.tile as tile
from concourse import bacc, mybir
from concourse._compat import with_exitstack
from concourse.bass_utils import run_bass_kernel_spmd

N_CORES = 8
BATCH, IN_DIM, OUT_DIM, G = 16384, 512, 512, 8
B_SHARD = BATCH // N_CORES          # 2048
B_CHUNK = 1024                      # batch columns processed per chunk
GRID_MIN, GRID_MAX, BN_EPS = -2.0, 2.0, 1e-3
DENOM = (GRID_MAX - GRID_MIN) / G   # 0.5
N_DT = IN_DIM // 128                # 4 d-tiles
K_SPLINE = N_DT * G                 # 32 spline K-chunks
K_BASE = N_DT                       # 4 base K-chunks
N_OSUB = OUT_DIM // 128             # 4 output partition tiles

F32 = mybir.dt.float32
F16 = mybir.dt.float16


def _grid_consts():
    grid = np.linspace(GRID_MIN, GRID_MAX, G, dtype=np.float32)
    c = (grid / np.float32(DENOM)).astype(np.float32)        # grid in u-units
    return c


@with_exitstack
def _body(ctx, tc, x16t, w_sp, w_b, params, out_t, b_shard, b_chunk):
    nc = tc.nc
    n_chunks = b_shard // b_chunk
    n_bh = b_chunk // 512            # 512-wide moving-operand slices
    k_total = K_SPLINE + K_BASE      # 36 K-chunks per PSUM tile
    KB = 6                           # K-chunks per PE block
    K_LAST = 30                      # kc >= K_LAST run per-bank, fused w/ epilogue
    W_SLICE = 8                      # spline weight K-chunks per SWDGE DMA

    const_pool = ctx.enter_context(tc.tile_pool(name="const", bufs=1))
    w_pool = ctx.enter_context(tc.tile_pool(name="w", bufs=1))
    xt_pool = ctx.enter_context(tc.tile_pool(name="xt", bufs=2 * N_DT))
    u_pool = ctx.enter_context(tc.tile_pool(name="u", bufs=3))
    # spline basis tiles: 32 per chunk stay resident through the chunk's
    # matmul phase; extra slots let the next chunk's production run ahead.
    basis_pool = ctx.enter_context(tc.tile_pool(name="basis", bufs=K_SPLINE + 12))
    relu_pool = ctx.enter_context(tc.tile_pool(name="relu", bufs=2 * N_DT))
    xb_pool = ctx.enter_context(tc.tile_pool(name="xb", bufs=2 * N_OSUB * 2))
    psum_pool = ctx.enter_context(
        tc.tile_pool(name="psum", bufs=8, space="PSUM"))
    out_pool = ctx.enter_context(tc.tile_pool(name="outs", bufs=4))

    # ---- warmup stationary + ACT table warm.  zeros via DVE memset is
    # ready ~2us before the old identity construction (gpsimd affine_select
    # behind the params DMA on the same engine), so the PE clock ramp
    # starts that much earlier ----
    zeros = const_pool.tile([128, 128], F16)
    nc.vector.memset(zeros, 0.0)
    scratch = const_pool.tile([128, 1], F16)
    nc.vector.memset(scratch, 0.0)
    # walrus inserts the ACT_TABLE_LOAD before this first ACTIVATE so it
    # overlaps the input DMAs
    nc.scalar.activation(out=scratch, in_=scratch,
                         func=mybir.ActivationFunctionType.Derivative_Erf)

    # ---- DMA order of battle (see module docstring).  xt[dt][ch] tiles for
    # BOTH chunks are allocated up front; dt0 + dt3 ride the sync queue,
    # dt1/dt2 are issued from the scalar engine inside the producer loop. ----
    params_sb = const_pool.tile([128, N_DT + N_DT + N_OSUB + G], F32)
    uscale_sb = params_sb[:, 0:N_DT]
    ushift_sb = params_sb[:, N_DT:2 * N_DT]
    bias_sb = params_sb[:, 2 * N_DT:2 * N_DT + N_OSUB]
    negc = params_sb[:, 2 * N_DT + N_OSUB:]

    xts = [[xt_pool.tile([128, b_chunk], F16, tag="xt",
                         name=f"xt{dt}_{ch}")
            for ch in range(n_chunks)] for dt in range(N_DT)]

    def xt_dma(eng, dt, ch):
        eng.dma_start(
            out=xts[dt][ch],
            in_=x16t[dt * 128:(dt + 1) * 128,
                     ch * b_chunk:(ch + 1) * b_chunk])

    # The two startup-critical 128-packet loads ride the fast sync queue
    # back-to-back: xt(dt0,ch0) lands ~10.2us (gates u -> basis ~11.5),
    # W[0:2] lands ~12.2us (gates the first matmul).  params (SWDGE
    # coalesces its tiny rows, done ~9.9) and W[2:6] (needed ~16us) go on
    # the SWDGE queue concurrently.
    # sync carries ONLY the two startup-critical 128-packet loads: a third
    # in-flight descriptor interleaves into the second's tail and drags its
    # completion by ~1.5us (observed).
    xt_dma(nc.sync, 0, 0)
    w_tile = w_pool.tile([128, K_SPLINE, OUT_DIM], F16)
    nc.sync.dma_start(out=w_tile[:, 0:2, :], in_=w_sp[:, 0:2, :])
    # Everything else streams on SWDGE in order of first need (params ~10.6,
    # kc2 ~16, kc6 ~23, u(dt1) ~24, kc14 ~31, u(dt2) ~35, ...).  Keeping the
    # xt[1..2] loads here (instead of scalar-engine descs) frees ~1.3us of
    # the ACT stream, which is the critical path through the dt0->dt1
    # basis handoff.
    wb_tile = w_pool.tile([128, K_BASE, OUT_DIM], F16)
    nc.gpsimd.dma_start(out=params_sb, in_=params)
    nc.gpsimd.dma_start(out=w_tile[:, 2:KB, :], in_=w_sp[:, 2:KB, :])
    nc.gpsimd.dma_start(out=w_tile[:, 6:14, :], in_=w_sp[:, 6:14, :])
    xt_dma(nc.gpsimd, 1, 0)
    xt_dma(nc.gpsimd, 1, 1)
    nc.gpsimd.dma_start(out=w_tile[:, 14:22, :], in_=w_sp[:, 14:22, :])
    xt_dma(nc.gpsimd, 2, 0)
    xt_dma(nc.gpsimd, 2, 1)
    nc.gpsimd.dma_start(out=w_tile[:, 22:30, :], in_=w_sp[:, 22:30, :])
    nc.gpsimd.dma_start(out=w_tile[:, 30:32, :], in_=w_sp[:, 30:32, :])
    nc.gpsimd.dma_start(out=wb_tile, in_=w_b)
    xt_dma(nc.gpsimd, 3, 0)
    xt_dma(nc.gpsimd, 3, 1)
    # xt(dt0,ch1) on the scalar queue.  Emitted after the scratch ACTIVATE:
    # a desc before the first ACT triggers a second ACT_TABLE_LOAD
    # (observed, +1.5us of ACT stream).
    xt_dma(nc.scalar, 0, 1)

    def emit_producers(ch):
        relus, basis = [], []
        for dt in range(N_DT):
            u = u_pool.tile([128, b_chunk], F32, tag="u")
            # chunk 0 / dt 0 is on the startup critical path: emit u and the
            # basis ACTs in halves so the first matmul's inputs exist as
            # soon as possible after the xt(dt0) DMA lands.
            halved = (ch == 0 and dt == 0)
            if halved:
                for h in range(0, b_chunk, 512):
                    nc.vector.tensor_scalar(
                        out=u[:, h:h + 512], in0=xts[dt][ch][:, h:h + 512],
                        scalar1=uscale_sb[:, dt:dt + 1],
                        scalar2=ushift_sb[:, dt:dt + 1],
                        op0=mybir.AluOpType.mult, op1=mybir.AluOpType.add,
                    )
            else:
                nc.vector.tensor_scalar(
                    out=u, in0=xts[dt][ch],
                    scalar1=uscale_sb[:, dt:dt + 1],
                    scalar2=ushift_sb[:, dt:dt + 1],
                    op0=mybir.AluOpType.mult, op1=mybir.AluOpType.add,
                )
            rl = relu_pool.tile([128, b_chunk], F16, tag="relu")
            nc.vector.tensor_scalar_max(out=rl, in0=xts[dt][ch], scalar1=0.0)
            relus.append(rl)
            for g in range(G):
                bt = basis_pool.tile([128, b_chunk], F16, tag="basis")
                # basis_g = sqrt(pi)/2 * d/dx erf(u - c_g); constant folded
                # into W_spline host-side.
                if halved:
                    for h in range(0, b_chunk, 512):
                        nc.scalar.activation(
                            out=bt[:, h:h + 512], in_=u[:, h:h + 512],
                            func=mybir.ActivationFunctionType.Derivative_Erf,
                            bias=negc[:, g:g + 1],
                        )
                else:
                    nc.scalar.activation(
                        out=bt, in_=u,
                        func=mybir.ActivationFunctionType.Derivative_Erf,
                        bias=negc[:, g:g + 1],
                    )
                basis.append(bt)
        return relus, basis

    def emit_xb(ch):
        # xb[osub,bh] = x_T + bias  (per-partition scalar add on fp16, 2x
        # DVE rate) -- consumed by the single-op epilogue.  Runs on DVE
        # cycles that are otherwise idle during the matmul stream; emitted
        # just before the chunk's final block so waiting on late xt tiles
        # never stalls the next chunk's u production.
        xbs = []
        for osub in range(N_OSUB):
            for bh in range(n_bh):
                xb = xb_pool.tile([128, 512], F16, tag="xb")
                nc.vector.tensor_scalar_add(
                    out=xb,
                    in0=xts[osub][ch][:, bh * 512:(bh + 1) * 512],
                    scalar1=bias_sb[:, osub:osub + 1])
                xbs.append(xb)
        return xbs

    def operands(kc, osub, relus, basis):
        if kc < K_SPLINE:
            return w_tile[:, kc, osub * 128:(osub + 1) * 128], basis[kc]
        dt = kc - K_SPLINE
        return wb_tile[:, dt, osub * 128:(osub + 1) * 128], relus[dt]

    def emit_main_blocks(ch, psums, relus, basis):
        # kc-outer: the PE touches a new basis tile only every
        # N_OSUB*n_bh matmuls (~1.7us), so late producers don't stall it.
        for kb in range(0, K_LAST, KB):
            for kc in range(kb, kb + KB):
                for osub in range(N_OSUB):
                    for bh in range(n_bh):
                        lhsT, rhs = operands(kc, osub, relus, basis)
                        nc.tensor.matmul(
                            psums[osub * n_bh + bh], lhsT=lhsT,
                            rhs=rhs[:, bh * 512:(bh + 1) * 512],
                            start=(kc == 0), stop=False)

    def emit_final_block(ch, psums, relus, basis, xbs):
        b0 = ch * b_chunk
        for osub in range(N_OSUB):
            for bh in range(n_bh):
                ps = psums[osub * n_bh + bh]
                for kc in range(K_LAST, k_total):
                    lhsT, rhs = operands(kc, osub, relus, basis)
                    nc.tensor.matmul(
                        ps, lhsT=lhsT, rhs=rhs[:, bh * 512:(bh + 1) * 512],
                        start=False, stop=(kc == k_total - 1))
                ot = out_pool.tile([128, 512], F16, tag="ot")
                xb = xbs[osub * n_bh + bh]
                last_tile = (ch == n_chunks - 1 and osub == N_OSUB - 1
                             and bh == n_bh - 1)
                orow = out_t[osub * 128:(osub + 1) * 128,
                             b0 + bh * 512:b0 + (bh + 1) * 512]
                if last_tile:
                    # the very last tile is the post-stream tail: halve the
                    # epilogue and fan the two stores across the scalar and
                    # sync queues so DVE time and descriptor time overlap
                    for h, eng in ((0, nc.scalar), (256, nc.sync)):
                        nc.vector.tensor_tensor(
                            out=ot[:, h:h + 256], in0=ps[:, h:h + 256],
                            in1=xb[:, h:h + 256], op=mybir.AluOpType.add)
                        eng.dma_start(out=orow[:, h:h + 256],
                                      in_=ot[:, h:h + 256])
                else:
                    nc.vector.tensor_tensor(
                        out=ot, in0=ps, in1=xb, op=mybir.AluOpType.add)
                    # last chunk's stores go out on the (by then idle)
                    # scalar queue so the final store doesn't wait behind
                    # the sync queue's drain of the earlier tiles
                    eng = nc.scalar if ch == n_chunks - 1 else nc.sync
                    eng.dma_start(out=orow, in_=ot)

    # Emission order keeps each engine's in-order stream free of
    # cross-chunk serialization: chunk ch+1's DVE/ACT producer ops are
    # emitted BEFORE chunk ch's final block + epilogue (which wait on ch's
    # last matmuls).
    def alloc_psums(ch):
        return [psum_pool.tile([128, 512], F32, tag="ps", name=f"ps{ch}_{i}")
                for i in range(N_OSUB * n_bh)]

    psums0 = alloc_psums(0)
    # PE warm-up: dependency-free matmuls into psums0[0] start the HAM clock
    # ramp (~3us of sustained PE activity to reach full clock) while the
    # first basis tiles are still being produced; the real kc==0 matmul has
    # start=True, which resets the bank, so the junk never reaches the
    # output.  Sized to bridge from zeros-ready (~7us) to first-basis-ready
    # (~12us) at the mid-pstate rate: an undershoot idles the PE and can
    # drop the HAM clock back to half speed (observed), so err long.
    N_WARM = 49
    for j in range(N_WARM):
        nc.tensor.matmul(psums0[0][:, 0:128], lhsT=zeros, rhs=zeros,
                         start=(j == 0), stop=(j == N_WARM - 1))

    prod = emit_producers(0)
    psums = psums0
    for ch in range(n_chunks):
        emit_main_blocks(ch, psums, *prod)
        cur_prod, cur_psums = prod, psums
        if ch + 1 < n_chunks:
            prod = emit_producers(ch + 1)
            psums = alloc_psums(ch + 1)
        emit_final_block(ch, cur_psums, *cur_prod, emit_xb(ch))


def build_program(b_shard=B_SHARD, b_chunk=B_CHUNK):
    nc = bacc.Bacc("TRN2", target_bir_lowering=False, debug=False,
                   num_devices=N_CORES)
    x16t = nc.dram_tensor("x16t", [IN_DIM, b_shard], F16,
                          kind="ExternalInput").ap()
    w_sp = nc.dram_tensor("w_sp", [128, K_SPLINE, OUT_DIM], F16,
                          kind="ExternalInput").ap()
    w_b = nc.dram_tensor("w_base", [128, K_BASE, OUT_DIM], F16,
                         kind="ExternalInput").ap()
    n_par = 2 * N_DT + N_OSUB + G
    params = nc.dram_tensor("params", [128, n_par], F32,
                            kind="ExternalInput").ap()
    out_t = nc.dram_tensor("out_t", [OUT_DIM, b_shard], F16,
                           kind="ExternalOutput").ap()
    with tile.TileContext(nc) as tc:
        _body(tc, x16t, w_sp, w_b, params, out_t, b_shard, b_chunk)
    nc.compile()
    return nc


def make_in_maps(x, gamma, beta, moving_mean, moving_var, W_spline, b_spline,
                 W_base, b_base, n_cores=N_CORES):
    """Host-side preprocessing + per-core input shards."""
    x = np.asarray(x, dtype=np.float32)
    gamma = np.asarray(gamma, dtype=np.float32)
    beta = np.asarray(beta, dtype=np.float32)
    moving_mean = np.asarray(moving_mean, dtype=np.float32)
    moving_var = np.asarray(moving_var, dtype=np.float32)
    W_spline = np.asarray(W_spline, dtype=np.float32)
    W_base = np.asarray(W_base, dtype=np.float32)
    b_spline = np.asarray(b_spline, dtype=np.float32)
    b_base = np.asarray(b_base, dtype=np.float32)

    scale = gamma / np.sqrt(moving_var + np.float32(BN_EPS))
    shift = beta - moving_mean * scale
    uscale = (scale / np.float32(DENOM)).astype(np.float32)
    ushift = (shift / np.float32(DENOM)).astype(np.float32)

    x16t = np.ascontiguousarray(x.T.astype(np.float16))  # [D, B]
    # K-order on chip is (dt, g, d_in): kc = dt*8+g covers d in
    # [dt*128, (dt+1)*128) at grid g.  W_spline rows are (d, g)-ordered.
    w_r = (W_spline.reshape(N_DT, 128, G, OUT_DIM)
           .transpose(0, 2, 1, 3)            # (dt, g, d_in, o)
           .reshape(K_SPLINE, 128, OUT_DIM)
           .transpose(1, 0, 2))              # (d_in, kc, o)
    w_sp = np.ascontiguousarray(w_r * np.float32(np.sqrt(np.pi) / 2.0)
                               ).astype(np.float16)
    w_b = np.ascontiguousarray(
        W_base.reshape(K_BASE, 128, OUT_DIM).transpose(1, 0, 2)
    ).astype(np.float16)
    bias_o = (b_spline + b_base).astype(np.float32)
    c = _grid_consts()
    params = np.empty((128, 2 * N_DT + N_OSUB + G), np.float32)
    params[:, 0:N_DT] = uscale.reshape(N_DT, 128).T
    params[:, N_DT:2 * N_DT] = ushift.reshape(N_DT, 128).T
    params[:, 2 * N_DT:2 * N_DT + N_OSUB] = bias_o.reshape(N_OSUB, 128).T
    params[:, 2 * N_DT + N_OSUB:] = -c[None, :]

    b_shard = x.shape[0] // n_cores
    return [
        {
            "x16t": np.ascontiguousarray(
                x16t[:, ci * b_shard:(ci + 1) * b_shard]),
            "w_sp": w_sp,
            "w_base": w_b,
            "params": params,
        }
        for ci in range(n_cores)
    ]


_PROGRAM = None


def kernel(x, gamma, beta, moving_mean, moving_var, W_spline, b_spline,
           W_base, b_base):
    global _PROGRAM
    if _PROGRAM is None:
        _PROGRAM = build_program()
    in_maps = make_in_maps(x, gamma, beta, moving_mean, moving_var,
                           W_spline, b_spline, W_base, b_base)
    res = run_bass_kernel_spmd(_PROGRAM, in_maps, core_ids=list(range(N_CORES)))
    out = np.concatenate(
        [np.ascontiguousarray(res.results[ci]["out_t"].T)
         for ci in range(N_CORES)], axis=0)
    return out.astype(np.float32)


# revision 40
# speedup vs baseline: 1.0040x; 1.0020x over previous
"""Trainium2 Bass kernel for EnhancedFastKANLayer.

Reference computation (B=16384, D=O=512, G=8 grids):
    x_norm = (x - mean) * rsqrt(var + eps) * gamma + beta          # BN inference
    basis[b,d,g] = exp(-((x_norm[b,d] - grid[g]) / denom)^2)       # RBF expansion
    out = basis.reshape(B, D*G) @ W_spline + b_spline
        + relu(x) @ W_base + b_base + x

Strategy:
  - Data parallel: batch 16384 sharded 8 ways (2048 rows/core); weights
    replicated. No collectives.
  - All on-chip compute happens in the transposed layout [feature, batch]:
    the output is produced as out_T [O, B_shard] and transposed back on the
    host. This makes BN/basis per-partition-scalar ops and lets the spline
    matmul consume basis tiles directly as the moving operand.
  - RBF via ScalarE Derivative_Erf: d/dx erf(x) = 2/sqrt(pi)*exp(-x^2), so
    basis_g = sqrt(pi)/2 * Derivative_Erf(u - c_g) -- ONE ACT op per grid
    (bias supplies -c_g), with the sqrt(pi)/2 constant folded into W_spline
    on the host.  HW-probed: rel err 1.3e-5, saturates cleanly to 0 for
    |x| > 6, no NaN/Inf out to |x|=24.
  - x is pre-cast to fp16 AND pre-transposed to [D, B_shard] on the host:
    fp16 keeps the BN input error at 2^-11 relative, and host-side
    transposition means every device DMA is a plain contiguous copy.
  - W_spline is pre-reordered on the host to K-order (dt, g, d_in) matching
    the order basis tiles are produced on chip, cast to fp16.
  - Matmul: out_T[o_sub, b] accumulates 36 matmuls per PSUM tile:
    32 spline K-chunks + 4 relu(x)@W_base K-chunks.  The "+ x" residual and
    the output bias are NOT matmuls: the epilogue is a single DVE
    tensor_tensor  out = psum + xb,  where xb = x_T + bias was precomputed
    on idle DVE cycles during the main matmul stream.  (A K=128 identity
    matmul costs the same 512 PE columns as a real K-chunk -- 1/37 of all
    PE time -- so it is strictly cheaper on the DVE.)
  - Startup choreography (trace-driven): the framework preamble ends ~7us;
    DMA queues move bytes from ~8.5us, and the ~358 GB/s per-core HBM
    bandwidth is SHARED across the three usable queues (sync HWDGE, scalar
    HWDGE, gpsimd SWDGE), so whatever streams first starves the rest.
    Order of battle:
      sync:    params (tiny) -> xt[dt0] ch0/ch1 -> W[6:32]+Wb (needed from
               ~24us) -> xt[dt3] pair (needed ~40us)
      gpsimd:  W[0:2], W[2:6] only (first PE block's weights, ~0.8 MB)
      scalar:  xt[dt1] pair / xt[dt2] pair, issued BETWEEN basis ACT groups
               (the ACT stream itself paces them; each lands well before
               its d-tile's basis is produced)
    so the startup-critical bytes (params + xt[dt0] + W[0:6]) have the HBM
    to themselves and the first real matmul can fire ~11.5-12.5us.
  - All matmul blocks are kc-outer (all 8 PSUM tiles at kc, then kc+1):
    the PE consumes one new basis tile per ~1.7us instead of one per
    ~0.2us burst, so it never outruns the ACT engine or the W stream.
  - Warmup matmuls (vs a memset zeros tile -- no identity needed) start as
    soon as the DVE memset lands (~7us), ramping the HAM clock throttle
    (full clock needs ~3us of sustained PE activity, and a PE idle gap can
    drop it back) while the first basis tiles are made.
  - Output is written fp16 (2^-11 relative is far inside the error budget),
    halving the tail DMA drain after the last matmul.
"""

import numpy as np
from contextlib import ExitStack

import concourse.bass as bass
import concourse.tile as tile
from concourse import bacc, mybir
from concourse._compat import with_exitstack
from concourse.bass_utils import run_bass_kernel_spmd

N_CORES = 8
BATCH, IN_DIM, OUT_DIM, G = 16384, 512, 512, 8
B_SHARD = BATCH // N_CORES          # 2048
B_CHUNK = 1024                      # batch columns processed per chunk
GRID_MIN, GRID_MAX, BN_EPS = -2.0, 2.0, 1e-3
DENOM = (GRID_MAX - GRID_MIN) / G   # 0.5
N_DT = IN_DIM // 128                # 4 d-tiles
K_SPLINE = N_DT * G                 # 32 spline K-chunks
K_BASE = N_DT                       # 4 base K-chunks
N_OSUB = OUT_DIM // 128             # 4 output partition tiles

F32 = mybir.dt.float32
F16 = mybir.dt.float16


def _grid_consts():
    grid = np.linspace(GRID_MIN, GRID_MAX, G, dtype=np.float32)
    c = (grid / np.float32(DENOM)).astype(np.float32)        # grid in u-units
    return c


@with_exitstack
def _body(ctx, tc, x16t, w_sp, w_b, params, out_t, b_shard, b_chunk):
    nc = tc.nc
    n_chunks = b_shard // b_chunk
    n_bh = b_chunk // 512            # 512-wide moving-operand slices
    k_total = K_SPLINE + K_BASE      # 36 K-chunks per PSUM tile
    KB = 6                           # K-chunks per PE block
    K_LAST = 30                      # kc >= K_LAST run per-bank, fused w/ epilogue
    W_SLICE = 8                      # spline weight K-chunks per SWDGE DMA

    const_pool = ctx.enter_context(tc.tile_pool(name="const", bufs=1))
    w_pool = ctx.enter_context(tc.tile_pool(name="w", bufs=1))
    xt_pool = ctx.enter_context(tc.tile_pool(name="xt", bufs=2 * N_DT))
    u_pool = ctx.enter_context(tc.tile_pool(name="u", bufs=3))
    # spline basis tiles: 32 per chunk stay resident through the chunk's
    # matmul phase; extra slots let the next chunk's production run ahead.
    basis_pool = ctx.enter_context(tc.tile_pool(name="basis", bufs=K_SPLINE + 12))
    relu_pool = ctx.enter_context(tc.tile_pool(name="relu", bufs=2 * N_DT))
    xb_pool = ctx.enter_context(tc.tile_pool(name="xb", bufs=2 * N_OSUB * 2))
    psum_pool = ctx.enter_context(
        tc.tile_pool(name="psum", bufs=8, space="PSUM"))
    out_pool = ctx.enter_context(tc.tile_pool(name="outs", bufs=4))

    # ---- warmup stationary + ACT table warm.  zeros via DVE memset is
    # ready ~2us before the old identity construction (gpsimd affine_select
    # behind the params DMA on the same engine), so the PE clock ramp
    # starts that much earlier ----
    # zeros on GpSimd: its user instructions start ~0.6us before Vector's
    # (observed), so the PE clock ramp starts that much earlier -- in 2 of 3
    # traced runs HAM reached full clock only AFTER the first real matmuls.
    zeros = const_pool.tile([128, 128], F16)
    nc.gpsimd.memset(zeros, 0.0)
    scratch = const_pool.tile([128, 1], F16)
    nc.vector.memset(scratch, 0.0)
    # walrus inserts the ACT_TABLE_LOAD before this first ACTIVATE so it
    # overlaps the input DMAs
    nc.scalar.activation(out=scratch, in_=scratch,
                         func=mybir.ActivationFunctionType.Derivative_Erf)

    # ---- DMA order of battle (see module docstring).  xt[dt][ch] tiles for
    # BOTH chunks are allocated up front; dt0 + dt3 ride the sync queue,
    # dt1/dt2 are issued from the scalar engine inside the producer loop. ----
    params_sb = const_pool.tile([128, N_DT + N_DT + N_OSUB + G], F32)
    uscale_sb = params_sb[:, 0:N_DT]
    ushift_sb = params_sb[:, N_DT:2 * N_DT]
    bias_sb = params_sb[:, 2 * N_DT:2 * N_DT + N_OSUB]
    negc = params_sb[:, 2 * N_DT + N_OSUB:]

    xts = [[xt_pool.tile([128, b_chunk], F16, tag="xt",
                         name=f"xt{dt}_{ch}")
            for ch in range(n_chunks)] for dt in range(N_DT)]

    def xt_dma(eng, dt, ch):
        eng.dma_start(
            out=xts[dt][ch],
            in_=x16t[dt * 128:(dt + 1) * 128,
                     ch * b_chunk:(ch + 1) * b_chunk])

    # The two startup-critical 128-packet loads ride the fast sync queue
    # back-to-back: xt(dt0,ch0) lands ~10.2us (gates u -> basis ~11.5),
    # W[0:2] lands ~12.2us (gates the first matmul).  params (SWDGE
    # coalesces its tiny rows, done ~9.9) and W[2:6] (needed ~16us) go on
    # the SWDGE queue concurrently.
    # sync carries ONLY the two startup-critical 128-packet loads: a third
    # in-flight descriptor interleaves into the second's tail and drags its
    # completion by ~1.5us (observed).
    xt_dma(nc.sync, 0, 0)
    w_tile = w_pool.tile([128, K_SPLINE, OUT_DIM], F16)
    nc.sync.dma_start(out=w_tile[:, 0:2, :], in_=w_sp[:, 0:2, :])
    # Everything else streams on SWDGE in order of first need (params ~10.6,
    # kc2 ~16, kc6 ~23, u(dt1) ~24, kc14 ~31, u(dt2) ~35, ...).  Keeping the
    # xt[1..2] loads here (instead of scalar-engine descs) frees ~1.3us of
    # the ACT stream, which is the critical path through the dt0->dt1
    # basis handoff.
    wb_tile = w_pool.tile([128, K_BASE, OUT_DIM], F16)
    nc.gpsimd.dma_start(out=params_sb, in_=params)
    nc.gpsimd.dma_start(out=w_tile[:, 2:KB, :], in_=w_sp[:, 2:KB, :])
    nc.gpsimd.dma_start(out=w_tile[:, 6:14, :], in_=w_sp[:, 6:14, :])
    xt_dma(nc.gpsimd, 1, 0)
    xt_dma(nc.gpsimd, 1, 1)
    nc.gpsimd.dma_start(out=w_tile[:, 14:22, :], in_=w_sp[:, 14:22, :])
    xt_dma(nc.gpsimd, 2, 0)
    xt_dma(nc.gpsimd, 2, 1)
    nc.gpsimd.dma_start(out=w_tile[:, 22:30, :], in_=w_sp[:, 22:30, :])
    nc.gpsimd.dma_start(out=w_tile[:, 30:32, :], in_=w_sp[:, 30:32, :])
    nc.gpsimd.dma_start(out=wb_tile, in_=w_b)
    xt_dma(nc.gpsimd, 3, 0)
    xt_dma(nc.gpsimd, 3, 1)
    # xt(dt0,ch1) on the scalar queue.  Emitted after the scratch ACTIVATE:
    # a desc before the first ACT triggers a second ACT_TABLE_LOAD
    # (observed, +1.5us of ACT stream).
    xt_dma(nc.scalar, 0, 1)

    def emit_producers(ch):
        relus, basis = [], []
        for dt in range(N_DT):
            u = u_pool.tile([128, b_chunk], F32, tag="u")
            # chunk 0 / dt 0 is on the startup critical path: emit u and the
            # basis ACTs in halves so the first matmul's inputs exist as
            # soon as possible after the xt(dt0) DMA lands.
            halved = (ch == 0 and dt == 0)
            if halved:
                for h in range(0, b_chunk, 512):
                    nc.vector.tensor_scalar(
                        out=u[:, h:h + 512], in0=xts[dt][ch][:, h:h + 512],
                        scalar1=uscale_sb[:, dt:dt + 1],
                        scalar2=ushift_sb[:, dt:dt + 1],
                        op0=mybir.AluOpType.mult, op1=mybir.AluOpType.add,
                    )
            else:
                nc.vector.tensor_scalar(
                    out=u, in0=xts[dt][ch],
                    scalar1=uscale_sb[:, dt:dt + 1],
                    scalar2=ushift_sb[:, dt:dt + 1],
                    op0=mybir.AluOpType.mult, op1=mybir.AluOpType.add,
                )
            rl = relu_pool.tile([128, b_chunk], F16, tag="relu")
            nc.vector.tensor_scalar_max(out=rl, in0=xts[dt][ch], scalar1=0.0)
            relus.append(rl)
            for g in range(G):
                bt = basis_pool.tile([128, b_chunk], F16, tag="basis")
                # basis_g = sqrt(pi)/2 * d/dx erf(u - c_g); constant folded
                # into W_spline host-side.
                if halved:
                    for h in range(0, b_chunk, 512):
                        nc.scalar.activation(
                            out=bt[:, h:h + 512], in_=u[:, h:h + 512],
                            func=mybir.ActivationFunctionType.Derivative_Erf,
                            bias=negc[:, g:g + 1],
                        )
                else:
                    nc.scalar.activation(
                        out=bt, in_=u,
                        func=mybir.ActivationFunctionType.Derivative_Erf,
                        bias=negc[:, g:g + 1],
                    )
                basis.append(bt)
        return relus, basis

    def emit_xb(ch):
        # xb[osub,bh] = x_T + bias  (per-partition scalar add on fp16, 2x
        # DVE rate) -- consumed by the single-op epilogue.  Runs on DVE
        # cycles that are otherwise idle during the matmul stream; emitted
        # just before the chunk's final block so waiting on late xt tiles
        # never stalls the next chunk's u production.
        xbs = []
        for osub in range(N_OSUB):
            for bh in range(n_bh):
                xb = xb_pool.tile([128, 512], F16, tag="xb")
                nc.vector.tensor_scalar_add(
                    out=xb,
                    in0=xts[osub][ch][:, bh * 512:(bh + 1) * 512],
                    scalar1=bias_sb[:, osub:osub + 1])
                xbs.append(xb)
        return xbs

    def operands(kc, osub, relus, basis):
        if kc < K_SPLINE:
            return w_tile[:, kc, osub * 128:(osub + 1) * 128], basis[kc]
        dt = kc - K_SPLINE
        return wb_tile[:, dt, osub * 128:(osub + 1) * 128], relus[dt]

    def emit_main_blocks(ch, psums, relus, basis):
        # kc-outer: the PE touches a new basis tile only every
        # N_OSUB*n_bh matmuls (~1.7us), so late producers don't stall it.
        for kb in range(0, K_LAST, KB):
            for kc in range(kb, kb + KB):
                for osub in range(N_OSUB):
                    for bh in range(n_bh):
                        lhsT, rhs = operands(kc, osub, relus, basis)
                        nc.tensor.matmul(
                            psums[osub * n_bh + bh], lhsT=lhsT,
                            rhs=rhs[:, bh * 512:(bh + 1) * 512],
                            start=(kc == 0), stop=False)

    def emit_final_block(ch, psums, relus, basis, xbs):
        b0 = ch * b_chunk
        for osub in range(N_OSUB):
            for bh in range(n_bh):
                ps = psums[osub * n_bh + bh]
                for kc in range(K_LAST, k_total):
                    lhsT, rhs = operands(kc, osub, relus, basis)
                    nc.tensor.matmul(
                        ps, lhsT=lhsT, rhs=rhs[:, bh * 512:(bh + 1) * 512],
                        start=False, stop=(kc == k_total - 1))
                ot = out_pool.tile([128, 512], F16, tag="ot")
                xb = xbs[osub * n_bh + bh]
                last_tile = (ch == n_chunks - 1 and osub == N_OSUB - 1
                             and bh == n_bh - 1)
                orow = out_t[osub * 128:(osub + 1) * 128,
                             b0 + bh * 512:b0 + (bh + 1) * 512]
                if last_tile:
                    # the very last tile is the post-stream tail: halve the
                    # epilogue and fan the two stores across the scalar and
                    # sync queues so DVE time and descriptor time overlap
                    for h, eng in ((0, nc.scalar), (256, nc.sync)):
                        nc.vector.tensor_tensor(
                            out=ot[:, h:h + 256], in0=ps[:, h:h + 256],
                            in1=xb[:, h:h + 256], op=mybir.AluOpType.add)
                        eng.dma_start(out=orow[:, h:h + 256],
                                      in_=ot[:, h:h + 256])
                else:
                    nc.vector.tensor_tensor(
                        out=ot, in0=ps, in1=xb, op=mybir.AluOpType.add)
                    # last chunk's stores go out on the (by then idle)
                    # scalar queue so the final store doesn't wait behind
                    # the sync queue's drain of the earlier tiles
                    eng = nc.scalar if ch == n_chunks - 1 else nc.sync
                    eng.dma_start(out=orow, in_=ot)

    # Emission order keeps each engine's in-order stream free of
    # cross-chunk serialization: chunk ch+1's DVE/ACT producer ops are
    # emitted BEFORE chunk ch's final block + epilogue (which wait on ch's
    # last matmuls).
    def alloc_psums(ch):
        return [psum_pool.tile([128, 512], F32, tag="ps", name=f"ps{ch}_{i}")
                for i in range(N_OSUB * n_bh)]

    psums0 = alloc_psums(0)
    # PE warm-up: dependency-free matmuls into psums0[0] start the HAM clock
    # ramp (~3us of sustained PE activity to reach full clock) while the
    # first basis tiles are still being produced; the real kc==0 matmul has
    # start=True, which resets the bank, so the junk never reaches the
    # output.  Sized to bridge from zeros-ready (~7us) to first-basis-ready
    # (~12us) at the mid-pstate rate: an undershoot idles the PE and can
    # drop the HAM clock back to half speed (observed), so err long.
    N_WARM = 54
    for j in range(N_WARM):
        nc.tensor.matmul(psums0[0][:, 0:128], lhsT=zeros, rhs=zeros,
                         start=(j == 0), stop=(j == N_WARM - 1))

    prod = emit_producers(0)
    psums = psums0
    for ch in range(n_chunks):
        emit_main_blocks(ch, psums, *prod)
        cur_prod, cur_psums = prod, psums
        if ch + 1 < n_chunks:
            prod = emit_producers(ch + 1)
            psums = alloc_psums(ch + 1)
        emit_final_block(ch, cur_psums, *cur_prod, emit_xb(ch))


def build_program(b_shard=B_SHARD, b_chunk=B_CHUNK):
    nc = bacc.Bacc("TRN2", target_bir_lowering=False, debug=False,
                   num_devices=N_CORES)
    x16t = nc.dram_tensor("x16t", [IN_DIM, b_shard], F16,
                          kind="ExternalInput").ap()
    w_sp = nc.dram_tensor("w_sp", [128, K_SPLINE, OUT_DIM], F16,
                          kind="ExternalInput").ap()
    w_b = nc.dram_tensor("w_base", [128, K_BASE, OUT_DIM], F16,
                         kind="ExternalInput").ap()
    n_par = 2 * N_DT + N_OSUB + G
    params = nc.dram_tensor("params", [128, n_par], F32,
                            kind="ExternalInput").ap()
    out_t = nc.dram_tensor("out_t", [OUT_DIM, b_shard], F16,
                           kind="ExternalOutput").ap()
    with tile.TileContext(nc) as tc:
        _body(tc, x16t, w_sp, w_b, params, out_t, b_shard, b_chunk)
    nc.compile()
    return nc


def make_in_maps(x, gamma, beta, moving_mean, moving_var, W_spline, b_spline,
                 W_base, b_base, n_cores=N_CORES):
    """Host-side preprocessing + per-core input shards."""
    x = np.asarray(x, dtype=np.float32)
    gamma = np.asarray(gamma, dtype=np.float32)
    beta = np.asarray(beta, dtype=np.float32)
    moving_mean = np.asarray(moving_mean, dtype=np.float32)
    moving_var = np.asarray(moving_var, dtype=np.float32)
    W_spline = np.asarray(W_spline, dtype=np.float32)
    W_base = np.asarray(W_base, dtype=np.float32)
    b_spline = np.asarray(b_spline, dtype=np.float32)
    b_base = np.asarray(b_base, dtype=np.float32)

    scale = gamma / np.sqrt(moving_var + np.float32(BN_EPS))
    shift = beta - moving_mean * scale
    uscale = (scale / np.float32(DENOM)).astype(np.float32)
    ushift = (shift / np.float32(DENOM)).astype(np.float32)

    x16t = np.ascontiguousarray(x.T.astype(np.float16))  # [D, B]
    # K-order on chip is (dt, g, d_in): kc = dt*8+g covers d in
    # [dt*128, (dt+1)*128) at grid g.  W_spline rows are (d, g)-ordered.
    w_r = (W_spline.reshape(N_DT, 128, G, OUT_DIM)
           .transpose(0, 2, 1, 3)            # (dt, g, d_in, o)
           .reshape(K_SPLINE, 128, OUT_DIM)
           .transpose(1, 0, 2))              # (d_in, kc, o)
    w_sp = np.ascontiguousarray(w_r * np.float32(np.sqrt(np.pi) / 2.0)
                               ).astype(np.float16)
    w_b = np.ascontiguousarray(
        W_base.reshape(K_BASE, 128, OUT_DIM).transpose(1, 0, 2)
    ).astype(np.float16)
    bias_o = (b_spline + b_base).astype(np.float32)
    c = _grid_consts()
    params = np.empty((128, 2 * N_DT + N_OSUB + G), np.float32)
    params[:, 0:N_DT] = uscale.reshape(N_DT, 128).T
    params[:, N_DT:2 * N_DT] = ushift.reshape(N_DT, 128).T
    params[:, 2 * N_DT:2 * N_DT + N_OSUB] = bias_o.reshape(N_OSUB, 128).T
    params[:, 2 * N_DT + N_OSUB:] = -c[None, :]

    b_shard = x.shape[0] // n_cores
    return [
        {
            "x16t": np.ascontiguousarray(
                x16t[:, ci * b_shard:(ci + 1) * b_shard]),
            "w_sp": w_sp,
            "w_base": w_b,
            "params": params,
        }
        for ci in range(n_cores)
    ]


_PROGRAM = None


def kernel(x, gamma, beta, moving_mean, moving_var, W_spline, b_spline,
           W_base, b_base):
    global _PROGRAM
    if _PROGRAM is None:
        _PROGRAM = build_program()
    in_maps = make_in_maps(x, gamma, beta, moving_mean, moving_var,
                           W_spline, b_spline, W_base, b_base)
    res = run_bass_kernel_spmd(_PROGRAM, in_maps, core_ids=list(range(N_CORES)))
    out = np.concatenate(
        [np.ascontiguousarray(res.results[ci]["out_t"].T)
         for ci in range(N_CORES)], axis=0)
    return out.astype(np.float32)


# revision 41
# speedup vs baseline: 1.0041x; 1.0002x over previous
"""Trainium2 Bass kernel for EnhancedFastKANLayer.

Reference computation (B=16384, D=O=512, G=8 grids):
    x_norm = (x - mean) * rsqrt(var + eps) * gamma + beta          # BN inference
    basis[b,d,g] = exp(-((x_norm[b,d] - grid[g]) / denom)^2)       # RBF expansion
    out = basis.reshape(B, D*G) @ W_spline + b_spline
        + relu(x) @ W_base + b_base + x

Strategy:
  - Data parallel: batch 16384 sharded 8 ways (2048 rows/core); weights
    replicated. No collectives.
  - All on-chip compute happens in the transposed layout [feature, batch]:
    the output is produced as out_T [O, B_shard] and transposed back on the
    host. This makes BN/basis per-partition-scalar ops and lets the spline
    matmul consume basis tiles directly as the moving operand.
  - RBF via ScalarE Derivative_Erf: d/dx erf(x) = 2/sqrt(pi)*exp(-x^2), so
    basis_g = sqrt(pi)/2 * Derivative_Erf(u - c_g) -- ONE ACT op per grid
    (bias supplies -c_g), with the sqrt(pi)/2 constant folded into W_spline
    on the host.  HW-probed: rel err 1.3e-5, saturates cleanly to 0 for
    |x| > 6, no NaN/Inf out to |x|=24.
  - x is pre-cast to fp16 AND pre-transposed to [D, B_shard] on the host:
    fp16 keeps the BN input error at 2^-11 relative, and host-side
    transposition means every device DMA is a plain contiguous copy.
  - W_spline is pre-reordered on the host to K-order (dt, g, d_in) matching
    the order basis tiles are produced on chip, cast to fp16.
  - Matmul: out_T[o_sub, b] accumulates 36 matmuls per PSUM tile:
    32 spline K-chunks + 4 relu(x)@W_base K-chunks.  The "+ x" residual and
    the output bias are NOT matmuls: the epilogue is a single DVE
    tensor_tensor  out = psum + xb,  where xb = x_T + bias was precomputed
    on idle DVE cycles during the main matmul stream.  (A K=128 identity
    matmul costs the same 512 PE columns as a real K-chunk -- 1/37 of all
    PE time -- so it is strictly cheaper on the DVE.)
  - Startup choreography (trace-driven): the framework preamble ends ~7us;
    DMA queues move bytes from ~8.5us, and the ~358 GB/s per-core HBM
    bandwidth is SHARED across the three usable queues (sync HWDGE, scalar
    HWDGE, gpsimd SWDGE), so whatever streams first starves the rest.
    Order of battle:
      sync:    params (tiny) -> xt[dt0] ch0/ch1 -> W[6:32]+Wb (needed from
               ~24us) -> xt[dt3] pair (needed ~40us)
      gpsimd:  W[0:2], W[2:6] only (first PE block's weights, ~0.8 MB)
      scalar:  xt[dt1] pair / xt[dt2] pair, issued BETWEEN basis ACT groups
               (the ACT stream itself paces them; each lands well before
               its d-tile's basis is produced)
    so the startup-critical bytes (params + xt[dt0] + W[0:6]) have the HBM
    to themselves and the first real matmul can fire ~11.5-12.5us.
  - All matmul blocks are kc-outer (all 8 PSUM tiles at kc, then kc+1):
    the PE consumes one new basis tile per ~1.7us instead of one per
    ~0.2us burst, so it never outruns the ACT engine or the W stream.
  - Warmup matmuls (vs a memset zeros tile -- no identity needed) start as
    soon as the DVE memset lands (~7us), ramping the HAM clock throttle
    (full clock needs ~3us of sustained PE activity, and a PE idle gap can
    drop it back) while the first basis tiles are made.
  - Output is written fp16 (2^-11 relative is far inside the error budget),
    halving the tail DMA drain after the last matmul.
"""

import numpy as np
from contextlib import ExitStack

import concourse.bass as bass
import concourse.tile as tile
from concourse import bacc, mybir
from concourse._compat import with_exitstack
from concourse.bass_utils import run_bass_kernel_spmd

N_CORES = 8
BATCH, IN_DIM, OUT_DIM, G = 16384, 512, 512, 8
B_SHARD = BATCH // N_CORES          # 2048
B_CHUNK = 1024                      # batch columns processed per chunk
GRID_MIN, GRID_MAX, BN_EPS = -2.0, 2.0, 1e-3
DENOM = (GRID_MAX - GRID_MIN) / G   # 0.5
N_DT = IN_DIM // 128                # 4 d-tiles
K_SPLINE = N_DT * G                 # 32 spline K-chunks
K_BASE = N_DT                       # 4 base K-chunks
N_OSUB = OUT_DIM // 128             # 4 output partition tiles

F32 = mybir.dt.float32
F16 = mybir.dt.float16


def _grid_consts():
    grid = np.linspace(GRID_MIN, GRID_MAX, G, dtype=np.float32)
    c = (grid / np.float32(DENOM)).astype(np.float32)        # grid in u-units
    return c


@with_exitstack
def _body(ctx, tc, x16t, w_sp, w_b, params, out_t, b_shard, b_chunk):
    nc = tc.nc
    n_chunks = b_shard // b_chunk
    n_bh = b_chunk // 512            # 512-wide moving-operand slices
    k_total = K_SPLINE + K_BASE      # 36 K-chunks per PSUM tile
    KB = 6                           # K-chunks per PE block
    K_LAST = 30                      # kc >= K_LAST run per-bank, fused w/ epilogue
    W_SLICE = 8                      # spline weight K-chunks per SWDGE DMA

    const_pool = ctx.enter_context(tc.tile_pool(name="const", bufs=1))
    w_pool = ctx.enter_context(tc.tile_pool(name="w", bufs=1))
    xt_pool = ctx.enter_context(tc.tile_pool(name="xt", bufs=2 * N_DT))
    u_pool = ctx.enter_context(tc.tile_pool(name="u", bufs=3))
    # spline basis tiles: 32 per chunk stay resident through the chunk's
    # matmul phase; extra slots let the next chunk's production run ahead.
    basis_pool = ctx.enter_context(tc.tile_pool(name="basis", bufs=K_SPLINE + 12))
    relu_pool = ctx.enter_context(tc.tile_pool(name="relu", bufs=2 * N_DT))
    xb_pool = ctx.enter_context(tc.tile_pool(name="xb", bufs=2 * N_OSUB * 2))
    psum_pool = ctx.enter_context(
        tc.tile_pool(name="psum", bufs=8, space="PSUM"))
    out_pool = ctx.enter_context(tc.tile_pool(name="outs", bufs=4))

    # ---- warmup stationary + ACT table warm.  zeros via DVE memset is
    # ready ~2us before the old identity construction (gpsimd affine_select
    # behind the params DMA on the same engine), so the PE clock ramp
    # starts that much earlier ----
    # zeros on GpSimd: its user instructions start ~0.6us before Vector's
    # (observed), so the PE clock ramp starts that much earlier -- in 2 of 3
    # traced runs HAM reached full clock only AFTER the first real matmuls.
    zeros = const_pool.tile([128, 128], F16)
    nc.gpsimd.memset(zeros, 0.0)
    scratch = const_pool.tile([128, 1], F16)
    nc.vector.memset(scratch, 0.0)
    # walrus inserts the ACT_TABLE_LOAD before this first ACTIVATE so it
    # overlaps the input DMAs
    nc.scalar.activation(out=scratch, in_=scratch,
                         func=mybir.ActivationFunctionType.Derivative_Erf)

    # ---- DMA order of battle (see module docstring).  xt[dt][ch] tiles for
    # BOTH chunks are allocated up front; dt0 + dt3 ride the sync queue,
    # dt1/dt2 are issued from the scalar engine inside the producer loop. ----
    params_sb = const_pool.tile([128, N_DT + N_DT + N_OSUB + G], F32)
    uscale_sb = params_sb[:, 0:N_DT]
    ushift_sb = params_sb[:, N_DT:2 * N_DT]
    bias_sb = params_sb[:, 2 * N_DT:2 * N_DT + N_OSUB]
    negc = params_sb[:, 2 * N_DT + N_OSUB:]

    xts = [[xt_pool.tile([128, b_chunk], F16, tag="xt",
                         name=f"xt{dt}_{ch}")
            for ch in range(n_chunks)] for dt in range(N_DT)]

    def xt_dma(eng, dt, ch):
        eng.dma_start(
            out=xts[dt][ch],
            in_=x16t[dt * 128:(dt + 1) * 128,
                     ch * b_chunk:(ch + 1) * b_chunk])

    # The two startup-critical 128-packet loads ride the fast sync queue
    # back-to-back: xt(dt0,ch0) lands ~10.2us (gates u -> basis ~11.5),
    # W[0:2] lands ~12.2us (gates the first matmul).  params (SWDGE
    # coalesces its tiny rows, done ~9.9) and W[2:6] (needed ~16us) go on
    # the SWDGE queue concurrently.
    # sync carries ONLY the two startup-critical 128-packet loads: a third
    # in-flight descriptor interleaves into the second's tail and drags its
    # completion by ~1.5us (observed).
    xt_dma(nc.sync, 0, 0)
    w_tile = w_pool.tile([128, K_SPLINE, OUT_DIM], F16)
    nc.sync.dma_start(out=w_tile[:, 0:2, :], in_=w_sp[:, 0:2, :])
    # Everything else streams on SWDGE in order of first need (params ~10.6,
    # kc2 ~16, kc6 ~23, u(dt1) ~24, kc14 ~31, u(dt2) ~35, ...).  Keeping the
    # xt[1..2] loads here (instead of scalar-engine descs) frees ~1.3us of
    # the ACT stream, which is the critical path through the dt0->dt1
    # basis handoff.
    wb_tile = w_pool.tile([128, K_BASE, OUT_DIM], F16)
    nc.gpsimd.dma_start(out=params_sb, in_=params)
    nc.gpsimd.dma_start(out=w_tile[:, 2:KB, :], in_=w_sp[:, 2:KB, :])
    nc.gpsimd.dma_start(out=w_tile[:, 6:14, :], in_=w_sp[:, 6:14, :])
    xt_dma(nc.gpsimd, 1, 0)
    xt_dma(nc.gpsimd, 1, 1)
    nc.gpsimd.dma_start(out=w_tile[:, 14:22, :], in_=w_sp[:, 14:22, :])
    xt_dma(nc.gpsimd, 2, 0)
    xt_dma(nc.gpsimd, 2, 1)
    nc.gpsimd.dma_start(out=w_tile[:, 22:30, :], in_=w_sp[:, 22:30, :])
    nc.gpsimd.dma_start(out=w_tile[:, 30:32, :], in_=w_sp[:, 30:32, :])
    nc.gpsimd.dma_start(out=wb_tile, in_=w_b)
    xt_dma(nc.gpsimd, 3, 0)
    xt_dma(nc.gpsimd, 3, 1)
    # xt(dt0,ch1) on the scalar queue.  Emitted after the scratch ACTIVATE:
    # a desc before the first ACT triggers a second ACT_TABLE_LOAD
    # (observed, +1.5us of ACT stream).
    xt_dma(nc.scalar, 0, 1)

    def emit_producers(ch):
        relus, basis = [], []
        for dt in range(N_DT):
            u = u_pool.tile([128, b_chunk], F32, tag="u")
            # chunk 0 / dt 0 is on the startup critical path: emit u and the
            # basis ACTs in halves so the first matmul's inputs exist as
            # soon as possible after the xt(dt0) DMA lands.
            halved = (ch == 0 and dt == 0)
            if halved:
                for h in range(0, b_chunk, 512):
                    nc.vector.tensor_scalar(
                        out=u[:, h:h + 512], in0=xts[dt][ch][:, h:h + 512],
                        scalar1=uscale_sb[:, dt:dt + 1],
                        scalar2=ushift_sb[:, dt:dt + 1],
                        op0=mybir.AluOpType.mult, op1=mybir.AluOpType.add,
                    )
            else:
                nc.vector.tensor_scalar(
                    out=u, in0=xts[dt][ch],
                    scalar1=uscale_sb[:, dt:dt + 1],
                    scalar2=ushift_sb[:, dt:dt + 1],
                    op0=mybir.AluOpType.mult, op1=mybir.AluOpType.add,
                )
            rl = relu_pool.tile([128, b_chunk], F16, tag="relu")
            nc.vector.tensor_scalar_max(out=rl, in0=xts[dt][ch], scalar1=0.0)
            relus.append(rl)
            for g in range(G):
                bt = basis_pool.tile([128, b_chunk], F16, tag="basis")
                # basis_g = sqrt(pi)/2 * d/dx erf(u - c_g); constant folded
                # into W_spline host-side.
                if halved:
                    for h in range(0, b_chunk, 512):
                        nc.scalar.activation(
                            out=bt[:, h:h + 512], in_=u[:, h:h + 512],
                            func=mybir.ActivationFunctionType.Derivative_Erf,
                            bias=negc[:, g:g + 1],
                        )
                else:
                    nc.scalar.activation(
                        out=bt, in_=u,
                        func=mybir.ActivationFunctionType.Derivative_Erf,
                        bias=negc[:, g:g + 1],
                    )
                basis.append(bt)
        return relus, basis

    def emit_xb(ch):
        # xb[osub,bh] = x_T + bias  (per-partition scalar add on fp16, 2x
        # DVE rate) -- consumed by the single-op epilogue.  Runs on DVE
        # cycles that are otherwise idle during the matmul stream; emitted
        # just before the chunk's final block so waiting on late xt tiles
        # never stalls the next chunk's u production.
        xbs = []
        for osub in range(N_OSUB):
            for bh in range(n_bh):
                xb = xb_pool.tile([128, 512], F16, tag="xb")
                nc.vector.tensor_scalar_add(
                    out=xb,
                    in0=xts[osub][ch][:, bh * 512:(bh + 1) * 512],
                    scalar1=bias_sb[:, osub:osub + 1])
                xbs.append(xb)
        return xbs

    def operands(kc, osub, relus, basis):
        if kc < K_SPLINE:
            return w_tile[:, kc, osub * 128:(osub + 1) * 128], basis[kc]
        dt = kc - K_SPLINE
        return wb_tile[:, dt, osub * 128:(osub + 1) * 128], relus[dt]

    def emit_main_blocks(ch, psums, relus, basis):
        # kc-outer: the PE touches a new basis tile only every
        # N_OSUB*n_bh matmuls (~1.7us), so late producers don't stall it.
        for kb in range(0, K_LAST, KB):
            for kc in range(kb, kb + KB):
                for osub in range(N_OSUB):
                    for bh in range(n_bh):
                        lhsT, rhs = operands(kc, osub, relus, basis)
                        nc.tensor.matmul(
                            psums[osub * n_bh + bh], lhsT=lhsT,
                            rhs=rhs[:, bh * 512:(bh + 1) * 512],
                            start=(kc == 0), stop=False)

    def emit_final_block(ch, psums, relus, basis, xbs):
        b0 = ch * b_chunk
        for osub in range(N_OSUB):
            for bh in range(n_bh):
                ps = psums[osub * n_bh + bh]
                for kc in range(K_LAST, k_total):
                    lhsT, rhs = operands(kc, osub, relus, basis)
                    nc.tensor.matmul(
                        ps, lhsT=lhsT, rhs=rhs[:, bh * 512:(bh + 1) * 512],
                        start=False, stop=(kc == k_total - 1))
                ot = out_pool.tile([128, 512], F16, tag="ot")
                xb = xbs[osub * n_bh + bh]
                last_tile = (ch == n_chunks - 1 and osub == N_OSUB - 1
                             and bh == n_bh - 1)
                orow = out_t[osub * 128:(osub + 1) * 128,
                             b0 + bh * 512:b0 + (bh + 1) * 512]
                if last_tile:
                    # the very last tile is the post-stream tail: halve the
                    # epilogue and fan the two stores across the scalar and
                    # sync queues so DVE time and descriptor time overlap
                    for h, eng in ((0, nc.scalar), (256, nc.sync)):
                        nc.vector.tensor_tensor(
                            out=ot[:, h:h + 256], in0=ps[:, h:h + 256],
                            in1=xb[:, h:h + 256], op=mybir.AluOpType.add)
                        eng.dma_start(out=orow[:, h:h + 256],
                                      in_=ot[:, h:h + 256])
                else:
                    nc.vector.tensor_tensor(
                        out=ot, in0=ps, in1=xb, op=mybir.AluOpType.add)
                    # last chunk's stores go out on the (by then idle)
                    # scalar queue so the final store doesn't wait behind
                    # the sync queue's drain of the earlier tiles
                    eng = nc.scalar if ch == n_chunks - 1 else nc.sync
                    eng.dma_start(out=orow, in_=ot)

    # Emission order keeps each engine's in-order stream free of
    # cross-chunk serialization: chunk ch+1's DVE/ACT producer ops are
    # emitted BEFORE chunk ch's final block + epilogue (which wait on ch's
    # last matmuls).
    def alloc_psums(ch):
        return [psum_pool.tile([128, 512], F32, tag="ps", name=f"ps{ch}_{i}")
                for i in range(N_OSUB * n_bh)]

    psums0 = alloc_psums(0)
    # PE warm-up: dependency-free matmuls into psums0[0] start the HAM clock
    # ramp (~3us of sustained PE activity to reach full clock) while the
    # first basis tiles are still being produced; the real kc==0 matmul has
    # start=True, which resets the bank, so the junk never reaches the
    # output.  Sized to bridge from zeros-ready (~7us) to first-basis-ready
    # (~12us) at the mid-pstate rate: an undershoot idles the PE and can
    # drop the HAM clock back to half speed (observed), so err long.
    N_WARM = 50
    for j in range(N_WARM):
        nc.tensor.matmul(psums0[0][:, 0:128], lhsT=zeros, rhs=zeros,
                         start=(j == 0), stop=(j == N_WARM - 1))

    prod = emit_producers(0)
    psums = psums0
    for ch in range(n_chunks):
        emit_main_blocks(ch, psums, *prod)
        cur_prod, cur_psums = prod, psums
        if ch + 1 < n_chunks:
            prod = emit_producers(ch + 1)
            psums = alloc_psums(ch + 1)
        emit_final_block(ch, cur_psums, *cur_prod, emit_xb(ch))


def build_program(b_shard=B_SHARD, b_chunk=B_CHUNK):
    nc = bacc.Bacc("TRN2", target_bir_lowering=False, debug=False,
                   num_devices=N_CORES)
    x16t = nc.dram_tensor("x16t", [IN_DIM, b_shard], F16,
                          kind="ExternalInput").ap()
    w_sp = nc.dram_tensor("w_sp", [128, K_SPLINE, OUT_DIM], F16,
                          kind="ExternalInput").ap()
    w_b = nc.dram_tensor("w_base", [128, K_BASE, OUT_DIM], F16,
                         kind="ExternalInput").ap()
    n_par = 2 * N_DT + N_OSUB + G
    params = nc.dram_tensor("params", [128, n_par], F32,
                            kind="ExternalInput").ap()
    out_t = nc.dram_tensor("out_t", [OUT_DIM, b_shard], F16,
                           kind="ExternalOutput").ap()
    with tile.TileContext(nc) as tc:
        _body(tc, x16t, w_sp, w_b, params, out_t, b_shard, b_chunk)
    nc.compile()
    return nc


def make_in_maps(x, gamma, beta, moving_mean, moving_var, W_spline, b_spline,
                 W_base, b_base, n_cores=N_CORES):
    """Host-side preprocessing + per-core input shards."""
    x = np.asarray(x, dtype=np.float32)
    gamma = np.asarray(gamma, dtype=np.float32)
    beta = np.asarray(beta, dtype=np.float32)
    moving_mean = np.asarray(moving_mean, dtype=np.float32)
    moving_var = np.asarray(moving_var, dtype=np.float32)
    W_spline = np.asarray(W_spline, dtype=np.float32)
    W_base = np.asarray(W_base, dtype=np.float32)
    b_spline = np.asarray(b_spline, dtype=np.float32)
    b_base = np.asarray(b_base, dtype=np.float32)

    scale = gamma / np.sqrt(moving_var + np.float32(BN_EPS))
    shift = beta - moving_mean * scale
    uscale = (scale / np.float32(DENOM)).astype(np.float32)
    ushift = (shift / np.float32(DENOM)).astype(np.float32)

    x16t = np.ascontiguousarray(x.T.astype(np.float16))  # [D, B]
    # K-order on chip is (dt, g, d_in): kc = dt*8+g covers d in
    # [dt*128, (dt+1)*128) at grid g.  W_spline rows are (d, g)-ordered.
    w_r = (W_spline.reshape(N_DT, 128, G, OUT_DIM)
           .transpose(0, 2, 1, 3)            # (dt, g, d_in, o)
           .reshape(K_SPLINE, 128, OUT_DIM)
           .transpose(1, 0, 2))              # (d_in, kc, o)
    w_sp = np.ascontiguousarray(w_r * np.float32(np.sqrt(np.pi) / 2.0)
                               ).astype(np.float16)
    w_b = np.ascontiguousarray(
        W_base.reshape(K_BASE, 128, OUT_DIM).transpose(1, 0, 2)
    ).astype(np.float16)
    bias_o = (b_spline + b_base).astype(np.float32)
    c = _grid_consts()
    params = np.empty((128, 2 * N_DT + N_OSUB + G), np.float32)
    params[:, 0:N_DT] = uscale.reshape(N_DT, 128).T
    params[:, N_DT:2 * N_DT] = ushift.reshape(N_DT, 128).T
    params[:, 2 * N_DT:2 * N_DT + N_OSUB] = bias_o.reshape(N_OSUB, 128).T
    params[:, 2 * N_DT + N_OSUB:] = -c[None, :]

    b_shard = x.shape[0] // n_cores
    return [
        {
            "x16t": np.ascontiguousarray(
                x16t[:, ci * b_shard:(ci + 1) * b_shard]),
            "w_sp": w_sp,
            "w_base": w_b,
            "params": params,
        }
        for ci in range(n_cores)
    ]


_PROGRAM = None


def kernel(x, gamma, beta, moving_mean, moving_var, W_spline, b_spline,
           W_base, b_base):
    global _PROGRAM
    if _PROGRAM is None:
        _PROGRAM = build_program()
    in_maps = make_in_maps(x, gamma, beta, moving_mean, moving_var,
                           W_spline, b_spline, W_base, b_base)
    res = run_bass_kernel_spmd(_PROGRAM, in_maps, core_ids=list(range(N_CORES)))
    out = np.concatenate(
        [np.ascontiguousarray(res.results[ci]["out_t"].T)
         for ci in range(N_CORES)], axis=0)
    return out.astype(np.float32)
